# revision 7
# baseline (speedup 1.0000x reference)
# Grouped GRU layer on 8 Trainium2 NeuronCores (one group per core).
#
# Problem: x [64, 500, 1024], 8 independent groups of (IG=128 -> HG=128) GRUs.
#   xp = einsum('btgi,gji->btgj', xg, W_ih) + b_ih        (input projection)
#   per step: hp = h @ W_hh[g].T + b_hh
#             r = sig(xr+hr); z = sig(xz+hz); n = tanh(xn + r*hn)
#             h = (1-z)*n + z*h
#
# Sharding: group g -> core g. Per-core layout is fully "transposed":
#   state h^T [HG=128 partitions, B=64 free], weights pre-transposed on host.
# Input projection matmuls write PSUM banks; the recurrence r/z matmuls
# accumulate on top (start=False), so xr+hr / xz+hz come out of PE for free.
# Sigmoid biases are folded in via the ACT per-partition bias operand, n-gate
# biases via scalar_tensor_tensor's per-partition scalar.

import numpy as np

B, T, IN, HID, G = 64, 500, 1024, 1024, 8
IG, HG = 128, 128

PSUM_STEPS = 8          # recurrence steps per PSUM bank chunk ([128, 8*64] fp32 = 1 bank)
RING_STEPS = 50         # output ring buffer length (steps) per DMA-out chunk

_CACHE = {}


def _build_program():
    import concourse.tile as tile
    from concourse import bacc, mybir

    f32 = mybir.dt.float32
    AF = mybir.ActivationFunctionType
    ALU = mybir.AluOpType

    nc = bacc.Bacc()
    xT = nc.declare_dram_parameter("xT", [IG, T * B], f32, isOutput=False)
    wih = nc.declare_dram_parameter("wih", [IG, 3 * HG], f32, isOutput=False)
    whh = nc.declare_dram_parameter("whh", [HG, 3 * HG], f32, isOutput=False)
    # per-partition bias columns: [r_bias, z_bias, b_ihn, b_hhn]
    biases = nc.declare_dram_parameter("biases", [HG, 4], f32, isOutput=False)
    y = nc.declare_dram_parameter("y", [HG, T * B], f32, isOutput=True)

    from contextlib import ExitStack

    with tile.TileContext(nc) as tc, ExitStack() as ctx:
        consts = ctx.enter_context(tc.tile_pool(name="consts", bufs=1))
        xpool = ctx.enter_context(tc.tile_pool(name="xin", bufs=3))
        # PSUM pools: input-projection(+accumulated recurrence) chunks, double buffered
        pr_pool = ctx.enter_context(tc.tile_pool(name="pr", bufs=2, space="PSUM"))
        pz_pool = ctx.enter_context(tc.tile_pool(name="pz", bufs=2, space="PSUM"))
        pn_pool = ctx.enter_context(tc.tile_pool(name="pn", bufs=2, space="PSUM"))
        hp_pool = ctx.enter_context(tc.tile_pool(name="hpn", bufs=2, space="PSUM"))
        work = ctx.enter_context(tc.tile_pool(name="work", bufs=4))
        ring_pool = ctx.enter_context(tc.tile_pool(name="ring", bufs=2))

        w_ih = consts.tile([IG, 3 * HG], f32)
        w_hh = consts.tile([HG, 3 * HG], f32)
        bias4 = consts.tile([HG, 4], f32)
        nc.sync.dma_start(out=w_ih, in_=wih[:, :])
        nc.sync.dma_start(out=w_hh, in_=whh[:, :])
        nc.sync.dma_start(out=bias4, in_=biases[:, :])
        bias_r = bias4[:, 0:1]
        bias_z = bias4[:, 1:2]
        b_ihn = bias4[:, 2:3]
        b_hhn = bias4[:, 3:4]

        h_init = consts.tile([HG, B], f32)
        nc.vector.memset(h_init, 0.0)

        n_chunks = (T + PSUM_STEPS - 1) // PSUM_STEPS

        h_prev = h_init
        ring = None
        for c in range(n_chunks):
            t0 = c * PSUM_STEPS
            steps = min(PSUM_STEPS, T - t0)
            nb = steps * B

            x_c = xpool.tile([IG, PSUM_STEPS * B], f32, tag="xc")
            nc.sync.dma_start(out=x_c[:, :nb], in_=xT[:, t0 * B : t0 * B + nb])

            p_r = pr_pool.tile([HG, PSUM_STEPS * B], f32, tag="pr")
            p_z = pz_pool.tile([HG, PSUM_STEPS * B], f32, tag="pz")
            p_n = pn_pool.tile([HG, PSUM_STEPS * B], f32, tag="pn")
            # input projections for the whole chunk: xp_j^T [HG, (t,b)]
            nc.tensor.matmul(p_r[:, :nb], w_ih[:, 0:HG], x_c[:, :nb],
                             start=True, stop=False, skip_group_check=True)
            nc.tensor.matmul(p_z[:, :nb], w_ih[:, HG:2 * HG], x_c[:, :nb],
                             start=True, stop=False, skip_group_check=True)
            nc.tensor.matmul(p_n[:, :nb], w_ih[:, 2 * HG:3 * HG], x_c[:, :nb],
                             start=True, stop=True, skip_group_check=True)

            for s in range(steps):
                t = t0 + s
                sl = slice(s * B, (s + 1) * B)
                if t % RING_STEPS == 0:
                    ring = ring_pool.tile([HG, RING_STEPS * B], f32, tag="ring")
                rsl = slice((t % RING_STEPS) * B, (t % RING_STEPS + 1) * B)

                # Recurrence matmuls, split by linearity:
                #   h_{t-1} = n + zh + zn'   (the three parts of the previous
                #   step's GRU update), each fed to PE as soon as available so
                #   the chain tail is only  tanh -> zn' -> matmul.
                hpn = hp_pool.tile([HG, B], f32, tag="hpn")
                if t > 0:
                    first_n = True
                    for rhs in prev_parts:
                        nc.tensor.matmul(p_r[:, sl], w_hh[:, 0:HG], rhs,
                                         start=False, stop=True,
                                         skip_group_check=True)
                        nc.tensor.matmul(p_z[:, sl], w_hh[:, HG:2 * HG], rhs,
                                         start=False, stop=True,
                                         skip_group_check=True)
                        nc.tensor.matmul(hpn, w_hh[:, 2 * HG:3 * HG], rhs,
                                         start=first_n, stop=True,
                                         skip_group_check=True)
                        first_n = False
                else:
                    # h_{-1} = 0: hp contribution is zero; just clear hpn
                    nc.tensor.matmul(hpn, w_hh[:, 2 * HG:3 * HG], h_init,
                                     start=True, stop=True,
                                     skip_group_check=True)

                r_sb = work.tile([HG, B], f32, tag="r")
                z_sb = work.tile([HG, B], f32, tag="z")
                nc.scalar.activation(r_sb, p_r[:, sl], AF.Sigmoid, bias=bias_r)
                nc.scalar.activation(z_sb, p_z[:, sl], AF.Sigmoid, bias=bias_z)

                # u = (hpn + b_hhn) * r
                u = work.tile([HG, B], f32, tag="u")
                nc.vector.scalar_tensor_tensor(
                    out=u, in0=hpn, scalar=b_hhn, in1=r_sb,
                    op0=ALU.add, op1=ALU.mult)
                # n_arg = (xpn + b_ihn) + u
                n_arg = work.tile([HG, B], f32, tag="narg")
                nc.vector.scalar_tensor_tensor(
                    out=n_arg, in0=p_n[:, sl], scalar=b_ihn, in1=u,
                    op0=ALU.add, op1=ALU.add)
                # zh = z * h_prev (off critical path, on GPSIMD)
                zh = work.tile([HG, B], f32, tag="zh")
                nc.gpsimd.tensor_mul(zh, z_sb, h_prev)
                n_sb = work.tile([HG, B], f32, tag="n")
                nc.scalar.activation(n_sb, n_arg, AF.Tanh)
                # zn' = -(n * z)   -- the only post-tanh op on the chain
                znm = work.tile([HG, B], f32, tag="znm")
                nc.vector.scalar_tensor_tensor(
                    out=znm, in0=n_sb, scalar=-1.0, in1=z_sb,
                    op0=ALU.mult, op1=ALU.mult)
                prev_parts = (zh, n_sb, znm)

                # h_new = n + zh + zn'  (output only; GPSIMD, off chain)
                w1 = work.tile([HG, B], f32, tag="w1")
                nc.gpsimd.tensor_add(w1, n_sb, zh)
                h_new = ring[:, rsl]
                nc.gpsimd.tensor_add(h_new, w1, znm)
                h_prev = h_new

                if (t + 1) % RING_STEPS == 0:
                    base = (t + 1 - RING_STEPS) * B
                    nc.sync.dma_start(out=y[:, base : base + RING_STEPS * B],
                                      in_=ring)
    nc.finalize()
    return nc


def _get_program():
    if "nc" not in _CACHE:
        _CACHE["nc"] = _build_program()
    return _CACHE["nc"]


def _prep_inputs(x, W_ih, W_hh, b_ih, b_hh):
    x = np.asarray(x, dtype=np.float32)
    W_ih = np.asarray(W_ih, dtype=np.float32)
    W_hh = np.asarray(W_hh, dtype=np.float32)
    b_ih = np.asarray(b_ih, dtype=np.float32)
    b_hh = np.asarray(b_hh, dtype=np.float32)

    # x [B,T,IN] -> per group [IG, T*B] with free index = t*B + b
    xg = x.reshape(B, T, G, IG)
    xT = np.ascontiguousarray(np.transpose(xg, (2, 3, 1, 0))).reshape(G, IG, T * B)

    wihT = np.ascontiguousarray(np.transpose(W_ih, (0, 2, 1)))  # [G, IG, 3HG]
    whhT = np.ascontiguousarray(np.transpose(W_hh, (0, 2, 1)))  # [G, HG, 3HG]

    biases = np.empty((G, HG, 4), np.float32)
    biases[:, :, 0] = b_ih[:, 0:HG] + b_hh[:, 0:HG]           # r
    biases[:, :, 1] = b_ih[:, HG:2 * HG] + b_hh[:, HG:2 * HG]  # z
    biases[:, :, 2] = b_ih[:, 2 * HG:3 * HG]                   # b_ihn
    biases[:, :, 3] = b_hh[:, 2 * HG:3 * HG]                   # b_hhn

    in_maps = []
    for g in range(G):
        in_maps.append({
            "xT": xT[g],
            "wih": wihT[g],
            "whh": whhT[g],
            "biases": biases[g],
        })
    return in_maps


def _assemble(results):
    out = np.empty((B, T, HID), np.float32)
    for g in range(G):
        yg = results[g]["y"].reshape(HG, T, B)          # [h, t, b]
        out[:, :, g * HG:(g + 1) * HG] = np.transpose(yg, (2, 1, 0))
    return out


def run(x, W_ih, W_hh, b_ih, b_hh, trace=False):
    from concourse.bass_utils import run_bass_kernel_spmd

    nc = _get_program()
    in_maps = _prep_inputs(x, W_ih, W_hh, b_ih, b_hh)
    res = run_bass_kernel_spmd(nc, in_maps, list(range(G)), trace=trace)
    return _assemble(res.results), res


def kernel(x, W_ih, W_hh, b_ih, b_hh):
    out, _ = run(x, W_ih, W_hh, b_ih, b_hh)
    return out


# revision 8
# speedup vs baseline: 1.0537x; 1.0537x over previous
# Grouped GRU layer on 8 Trainium2 NeuronCores (one group per core).
#
# Problem: x [64, 500, 1024], 8 independent groups of (IG=128 -> HG=128) GRUs.
#   xp = einsum('btgi,gji->btgj', xg, W_ih) + b_ih        (input projection)
#   per step: hp = h @ W_hh[g].T + b_hh
#             r = sig(xr+hr); z = sig(xz+hz); n = tanh(xn + r*hn)
#             h = (1-z)*n + z*h
#
# Sharding: group g -> core g. Per-core layout is fully "transposed":
#   state h^T [HG=128 partitions, B=64 free], weights pre-transposed on host.
# Input projection matmuls write PSUM banks; the recurrence r/z matmuls
# accumulate on top (start=False), so xr+hr / xz+hz come out of PE for free.
# Sigmoid biases are folded in via the ACT per-partition bias operand, n-gate
# biases via scalar_tensor_tensor's per-partition scalar.

import numpy as np

B, T, IN, HID, G = 64, 500, 1024, 1024, 8
IG, HG = 128, 128

PSUM_STEPS = 8          # recurrence steps per PSUM bank chunk ([128, 8*64] fp32 = 1 bank)
RING_STEPS = 50         # output ring buffer length (steps) per DMA-out chunk

_CACHE = {}


def _build_program():
    import concourse.tile as tile
    from concourse import bacc, mybir

    f32 = mybir.dt.float32
    AF = mybir.ActivationFunctionType
    ALU = mybir.AluOpType

    nc = bacc.Bacc()
    xT = nc.declare_dram_parameter("xT", [IG, T * B], f32, isOutput=False)
    wih = nc.declare_dram_parameter("wih", [IG, 3 * HG], f32, isOutput=False)
    whh = nc.declare_dram_parameter("whh", [HG, 3 * HG], f32, isOutput=False)
    # per-partition bias columns: [r_bias, z_bias, b_ihn, b_hhn]
    biases = nc.declare_dram_parameter("biases", [HG, 4], f32, isOutput=False)
    y = nc.declare_dram_parameter("y", [HG, T * B], f32, isOutput=True)

    from contextlib import ExitStack

    with tile.TileContext(nc) as tc, ExitStack() as ctx:
        consts = ctx.enter_context(tc.tile_pool(name="consts", bufs=1))
        xpool = ctx.enter_context(tc.tile_pool(name="xin", bufs=3))
        # PSUM pools: input-projection(+accumulated recurrence) chunks, double buffered
        pr_pool = ctx.enter_context(tc.tile_pool(name="pr", bufs=2, space="PSUM"))
        pz_pool = ctx.enter_context(tc.tile_pool(name="pz", bufs=2, space="PSUM"))
        pn_pool = ctx.enter_context(tc.tile_pool(name="pn", bufs=2, space="PSUM"))
        hp_pool = ctx.enter_context(tc.tile_pool(name="hpn", bufs=2, space="PSUM"))
        work = ctx.enter_context(tc.tile_pool(name="work", bufs=4))
        ring_pool = ctx.enter_context(tc.tile_pool(name="ring", bufs=2))

        w_ih = consts.tile([IG, 3 * HG], f32)
        w_hh = consts.tile([HG, 3 * HG], f32)
        bias4 = consts.tile([HG, 4], f32)
        nc.sync.dma_start(out=w_ih, in_=wih[:, :])
        nc.sync.dma_start(out=w_hh, in_=whh[:, :])
        nc.sync.dma_start(out=bias4, in_=biases[:, :])
        bias_r = bias4[:, 0:1]
        bias_z = bias4[:, 1:2]
        b_ihn = bias4[:, 2:3]
        b_hhn = bias4[:, 3:4]

        h_init = consts.tile([HG, B], f32)
        nc.vector.memset(h_init, 0.0)

        n_chunks = (T + PSUM_STEPS - 1) // PSUM_STEPS

        h_prev = h_init
        ring = None
        for c in range(n_chunks):
            t0 = c * PSUM_STEPS
            steps = min(PSUM_STEPS, T - t0)
            nb = steps * B

            x_c = xpool.tile([IG, PSUM_STEPS * B], f32, tag="xc")
            nc.sync.dma_start(out=x_c[:, :nb], in_=xT[:, t0 * B : t0 * B + nb])

            p_r = pr_pool.tile([HG, PSUM_STEPS * B], f32, tag="pr")
            p_z = pz_pool.tile([HG, PSUM_STEPS * B], f32, tag="pz")
            p_n = pn_pool.tile([HG, PSUM_STEPS * B], f32, tag="pn")
            # input projections for the whole chunk: xp_j^T [HG, (t,b)]
            nc.tensor.matmul(p_r[:, :nb], w_ih[:, 0:HG], x_c[:, :nb],
                             start=True, stop=False, skip_group_check=True)
            nc.tensor.matmul(p_z[:, :nb], w_ih[:, HG:2 * HG], x_c[:, :nb],
                             start=True, stop=False, skip_group_check=True)
            nc.tensor.matmul(p_n[:, :nb], w_ih[:, 2 * HG:3 * HG], x_c[:, :nb],
                             start=True, stop=True, skip_group_check=True)
            xn_sb = xpool.tile([HG, PSUM_STEPS * B], f32, tag="xnsb")
            nc.scalar.activation(xn_sb[:, :nb], p_n[:, :nb], AF.Copy)

            for s in range(steps):
                t = t0 + s
                sl = slice(s * B, (s + 1) * B)
                if t % RING_STEPS == 0:
                    ring = ring_pool.tile([HG, RING_STEPS * B], f32, tag="ring")
                rsl = slice((t % RING_STEPS) * B, (t % RING_STEPS + 1) * B)

                # Recurrence matmuls, split by linearity:
                #   h_{t-1} = n + zh + zn'   (the three parts of the previous
                #   step's GRU update), each fed to PE as soon as available so
                #   the chain tail is only  tanh -> zn' -> matmul.
                hpn = hp_pool.tile([HG, B], f32, tag="hpn")
                if t > 0:
                    first_n = True
                    for rhs in prev_parts:
                        nc.tensor.matmul(p_r[:, sl], w_hh[:, 0:HG], rhs,
                                         start=False, stop=True,
                                         skip_group_check=True)
                        nc.tensor.matmul(p_z[:, sl], w_hh[:, HG:2 * HG], rhs,
                                         start=False, stop=True,
                                         skip_group_check=True)
                        nc.tensor.matmul(hpn, w_hh[:, 2 * HG:3 * HG], rhs,
                                         start=first_n, stop=True,
                                         skip_group_check=True)
                        first_n = False
                else:
                    # h_{-1} = 0: hp contribution is zero; just clear hpn
                    nc.tensor.matmul(hpn, w_hh[:, 2 * HG:3 * HG], h_init,
                                     start=True, stop=True,
                                     skip_group_check=True)

                r_sb = work.tile([HG, B], f32, tag="r")
                z_sb = work.tile([HG, B], f32, tag="z")
                nc.scalar.activation(r_sb, p_r[:, sl], AF.Sigmoid, bias=bias_r)
                nc.scalar.activation(z_sb, p_z[:, sl], AF.Sigmoid, bias=bias_z)

                # u = (hpn + b_hhn) * r
                u = work.tile([HG, B], f32, tag="u")
                nc.vector.scalar_tensor_tensor(
                    out=u, in0=hpn, scalar=b_hhn, in1=r_sb,
                    op0=ALU.add, op1=ALU.mult)
                # n_arg = (xpn + b_ihn) + u
                n_arg = work.tile([HG, B], f32, tag="narg")
                nc.vector.scalar_tensor_tensor(
                    out=n_arg, in0=xn_sb[:, sl], scalar=b_ihn, in1=u,
                    op0=ALU.add, op1=ALU.add)
                # zh = z * h_prev (off critical path, on GPSIMD)
                zh = work.tile([HG, B], f32, tag="zh")
                nc.gpsimd.tensor_mul(zh, z_sb, h_prev)
                n_sb = work.tile([HG, B], f32, tag="n")
                nc.scalar.activation(n_sb, n_arg, AF.Tanh)
                # zn' = -(n * z)   -- the only post-tanh op on the chain
                znm = work.tile([HG, B], f32, tag="znm")
                nc.vector.scalar_tensor_tensor(
                    out=znm, in0=n_sb, scalar=-1.0, in1=z_sb,
                    op0=ALU.mult, op1=ALU.mult)
                prev_parts = (zh, n_sb, znm)

                # h_new = n + zh + zn'  (output only; GPSIMD, off chain)
                w1 = work.tile([HG, B], f32, tag="w1")
                nc.gpsimd.tensor_add(w1, n_sb, zh)
                h_new = ring[:, rsl]
                nc.gpsimd.tensor_add(h_new, w1, znm)
                h_prev = h_new

                if (t + 1) % RING_STEPS == 0:
                    base = (t + 1 - RING_STEPS) * B
                    nc.sync.dma_start(out=y[:, base : base + RING_STEPS * B],
                                      in_=ring)
    nc.finalize()
    return nc


def _get_program():
    if "nc" not in _CACHE:
        _CACHE["nc"] = _build_program()
    return _CACHE["nc"]


def _prep_inputs(x, W_ih, W_hh, b_ih, b_hh):
    x = np.asarray(x, dtype=np.float32)
    W_ih = np.asarray(W_ih, dtype=np.float32)
    W_hh = np.asarray(W_hh, dtype=np.float32)
    b_ih = np.asarray(b_ih, dtype=np.float32)
    b_hh = np.asarray(b_hh, dtype=np.float32)

    # x [B,T,IN] -> per group [IG, T*B] with free index = t*B + b
    xg = x.reshape(B, T, G, IG)
    xT = np.ascontiguousarray(np.transpose(xg, (2, 3, 1, 0))).reshape(G, IG, T * B)

    wihT = np.ascontiguousarray(np.transpose(W_ih, (0, 2, 1)))  # [G, IG, 3HG]
    whhT = np.ascontiguousarray(np.transpose(W_hh, (0, 2, 1)))  # [G, HG, 3HG]

    biases = np.empty((G, HG, 4), np.float32)
    biases[:, :, 0] = b_ih[:, 0:HG] + b_hh[:, 0:HG]           # r
    biases[:, :, 1] = b_ih[:, HG:2 * HG] + b_hh[:, HG:2 * HG]  # z
    biases[:, :, 2] = b_ih[:, 2 * HG:3 * HG]                   # b_ihn
    biases[:, :, 3] = b_hh[:, 2 * HG:3 * HG]                   # b_hhn

    in_maps = []
    for g in range(G):
        in_maps.append({
            "xT": xT[g],
            "wih": wihT[g],
            "whh": whhT[g],
            "biases": biases[g],
        })
    return in_maps


def _assemble(results):
    out = np.empty((B, T, HID), np.float32)
    for g in range(G):
        yg = results[g]["y"].reshape(HG, T, B)          # [h, t, b]
        out[:, :, g * HG:(g + 1) * HG] = np.transpose(yg, (2, 1, 0))
    return out


def run(x, W_ih, W_hh, b_ih, b_hh, trace=False):
    from concourse.bass_utils import run_bass_kernel_spmd

    nc = _get_program()
    in_maps = _prep_inputs(x, W_ih, W_hh, b_ih, b_hh)
    res = run_bass_kernel_spmd(nc, in_maps, list(range(G)), trace=trace)
    return _assemble(res.results), res


def kernel(x, W_ih, W_hh, b_ih, b_hh):
    out, _ = run(x, W_ih, W_hh, b_ih, b_hh)
    return out
